# revision 7
# baseline (speedup 1.0000x reference)
"""Trainium2 Bass kernel for nn_ANO_VQC_Model (14-qubit VQC, batch 512).

Math: the circuit's state, viewed as a 128x128 matrix M (rows = qubits 0-6,
cols = qubits 7-13), starts as a real rank-1 outer product u v^T (RY layer on
|+>^14 gives a real product state) and each entangling layer acts as
    M' = A0 M B0^T + A1 M B1^T
(only CNOT(6,7) couples rows and cols; it splits into 2 terms via projectors
on qubit 6).  So the state stays factored: L <- [A0 L | A1 L],
R <- [B0 R | B1 R], M = L R^T with rank <= 64 after 6 layers.  Everything is
real f32.  The two requested expectation values are
    e_q = sum( (L^T G_q L) * (R^T R) ),  G_q = Re(H_q) (x) I_16  (row space).

Sharding: pure data parallel, 64 batch elements per core on 8 cores.
"""

import os
import sys

import numpy as np

for _p in ("/opt/trn_rl_repo", "/root/.axon_site/_ro/trn_rl_repo"):
    if os.path.isdir(_p) and _p not in sys.path:
        sys.path.append(_p)

import concourse.bass as bass
import concourse.mybir as mybir
import concourse.tile as tile
from concourse import bacc
from concourse.bass_utils import run_bass_kernel_spmd

N_CORES = 8
BATCH = 512
BPC = BATCH // N_CORES  # 64
NQ = 14
DEPTH = 6
DA = 128  # row space (qubits 0-6)
DB = 128  # col space (qubits 7-13)

F32 = mybir.dt.float32
# dtype used for the matmul input tensors (weights / L / R / P buffers)
MM_DT = mybir.dt.bfloat16

_nc_cache = {}


# ----------------------------------------------------------------------------
# Host-side preprocessing (input-dependent constant folding)
# ----------------------------------------------------------------------------

def _ry(theta):
    c, s = np.cos(theta / 2), np.sin(theta / 2)
    return np.array([[c, -s], [s, c]], dtype=np.float64)


_CNOT = np.array(
    [[1, 0, 0, 0], [0, 1, 0, 0], [0, 0, 0, 1], [0, 0, 1, 0]], dtype=np.float64
)


def _kron_list(mats):
    out = mats[0]
    for m in mats[1:]:
        out = np.kron(out, m)
    return out


def _cnot_on(n, ctrl):
    mats, q = [], 0
    while q < n:
        if q == ctrl:
            mats.append(_CNOT)
            q += 2
        else:
            mats.append(np.eye(2))
            q += 1
    return _kron_list(mats)


def _layer_mats(theta_k):
    """A0, A1 (row ops) and B0, B1 (col ops) for one entangling layer."""
    C_evenA = _cnot_on(7, 0) @ _cnot_on(7, 2) @ _cnot_on(7, 4)
    C_oddA = _cnot_on(7, 1) @ _cnot_on(7, 3) @ _cnot_on(7, 5)
    R_A = _kron_list([_ry(theta_k[w]) for w in range(7)])
    C_evenB = _cnot_on(7, 1) @ _cnot_on(7, 3) @ _cnot_on(7, 5)
    C_oddB = _cnot_on(7, 0) @ _cnot_on(7, 2) @ _cnot_on(7, 4)
    R_B = _kron_list([_ry(theta_k[7 + w]) for w in range(7)])
    rows = np.arange(DA)
    P0 = np.diag((rows % 2 == 0).astype(np.float64))
    P1 = np.diag((rows % 2 == 1).astype(np.float64))
    S = np.zeros((DB, DB))
    S[: DB // 2, DB // 2:] = np.eye(DB // 2)
    S[DB // 2:, : DB // 2] = np.eye(DB // 2)
    A0 = R_A @ C_oddA @ P0 @ C_evenA
    A1 = R_A @ C_oddA @ P1 @ C_evenA
    B0 = R_B @ C_oddB @ C_evenB
    B1 = R_B @ C_oddB @ S @ C_evenB
    return A0, A1, B0, B1


def _measure_mats(A, B, D):
    """G_q = Re(H_q) expanded to the 128-dim row space, q = 0, 1."""
    NLOC = 8
    rows_t, cols_t = np.tril_indices(NLOC, -1)
    Gs = []
    for q in range(2):
        tri = np.zeros((NLOC, NLOC))
        tri[rows_t, cols_t] = A[q]
        h = tri + np.diag(np.concatenate([D[q][1:], [0.0]]))
        Hr = h + h.T
        if q == 0:
            G = np.kron(Hr, np.eye(16))  # wires 0,1,2 -> row bits 0-2
        else:
            G = np.kron(np.kron(np.eye(2), Hr), np.eye(8))  # wires 1,2,3
        Gs.append(G)
    return np.stack(Gs)


def _host_prep(X, theta, A, B, D):
    X = np.asarray(X, dtype=np.float64)
    theta = np.asarray(theta, dtype=np.float64)
    A = np.asarray(A, dtype=np.float64)
    B = np.asarray(B, dtype=np.float64)
    D = np.asarray(D, dtype=np.float64)
    nb = X.shape[0]
    c, s = np.cos(X / 2), np.sin(X / 2)
    v0 = (c - s) / np.sqrt(2.0)
    v1 = (c + s) / np.sqrt(2.0)

    def kron_side(ws):
        out = np.ones((nb, 1))
        for w in ws:
            pair = np.stack([v0[:, w], v1[:, w]], axis=1)
            out = (out[:, :, None] * pair[:, None, :]).reshape(nb, -1)
        return out

    U = kron_side(range(7))  # (B, 128)
    V = kron_side(range(7, 14))
    AT = np.empty((2 * DEPTH, DA, DA))
    BT = np.empty((2 * DEPTH, DB, DB))
    for k in range(DEPTH):
        A0, A1, B0, B1 = _layer_mats(theta[k])
        AT[2 * k + 0] = A0.T  # lhsT layout: out = lhsT.T @ rhs
        AT[2 * k + 1] = A1.T
        BT[2 * k + 0] = B0.T
        BT[2 * k + 1] = B1.T
    G = _measure_mats(A, B, D)  # (2, 128, 128), symmetric
    return U, V, AT, BT, G


# ----------------------------------------------------------------------------
# Device kernel
# ----------------------------------------------------------------------------

def _build_nc():
    nc = bacc.Bacc("TRN2", target_bir_lowering=False, debug=False)

    ut_d = nc.declare_dram_parameter("ut", [DA, BPC], MM_DT, isOutput=False)
    vt_d = nc.declare_dram_parameter("vt", [DB, BPC], MM_DT, isOutput=False)
    at_d = nc.declare_dram_parameter("at", [2 * DEPTH, DA, DA], MM_DT, isOutput=False)
    bt_d = nc.declare_dram_parameter("bt", [2 * DEPTH, DB, DB], MM_DT, isOutput=False)
    g_d = nc.declare_dram_parameter("g", [2, DA, DA], MM_DT, isOutput=False)
    out_d = nc.declare_dram_parameter("out", [1, 2 * BPC], F32, isOutput=True)

    with tile.TileContext(nc) as tc:
        with (
            tc.tile_pool(name="w", bufs=1) as wpool,
            tc.tile_pool(name="state", bufs=1) as spool,
            tc.tile_pool(name="grp", bufs=2) as gpool,
            tc.tile_pool(name="ps", bufs=2, space="PSUM") as pspool,
        ):
            aw = wpool.tile([DA, 2 * DEPTH * DA], MM_DT, tag="aw")
            bw = wpool.tile([DB, 2 * DEPTH * DB], MM_DT, tag="bw")
            gw = wpool.tile([DA, 2 * DA], MM_DT, tag="gw")
            ut = wpool.tile([DA, BPC], MM_DT, tag="ut")
            vt = wpool.tile([DB, BPC], MM_DT, tag="vt")
            ones = wpool.tile([64, 1], F32, tag="ones")

            # ut/vt first (layer-1 matmuls need them), one layer-pair of
            # weights per DMA, split across the two HWDGE queues (sync+scalar)
            nc.sync.dma_start(out=ut[:], in_=ut_d[:, :])
            nc.scalar.dma_start(out=vt[:], in_=vt_d[:, :])
            for k in range(DEPTH):
                src_a = at_d[2 * k:2 * k + 2].rearrange("i p m -> p i m")
                dst_a = aw[:, k * 256:(k + 1) * 256].rearrange(
                    "p (i m) -> p i m", i=2
                )
                nc.sync.dma_start(out=dst_a, in_=src_a)
                src_b = bt_d[2 * k:2 * k + 2].rearrange("i p m -> p i m")
                dst_b = bw[:, k * 256:(k + 1) * 256].rearrange(
                    "p (i m) -> p i m", i=2
                )
                nc.scalar.dma_start(out=dst_b, in_=src_b)
            nc.sync.dma_start(
                out=gw[:].rearrange("p (q m) -> p q m", q=2),
                in_=g_d[:].rearrange("q p m -> p q m"),
            )
            nc.vector.memset(ones[:], 1.0)

            Ltmp = spool.tile([DA, 32 * BPC], MM_DT, tag="Ltmp")
            Lbuf = spool.tile([DA, 64 * BPC], MM_DT, tag="Lbuf")
            Rtmp = spool.tile([DB, 32 * BPC], MM_DT, tag="Rtmp")
            Rbuf = spool.tile([DB, 64 * BPC], MM_DT, tag="Rbuf")
            Pbuf = spool.tile([DA, 2 * 64 * BPC], MM_DT, tag="Pbuf")
            Z = spool.tile([64, 2 * BPC], F32, tag="Z")
            esb = spool.tile([1, 2 * BPC], F32, tag="esb")

            def recursion(w_tile, init_ap, buf_tmp, buf_big):
                cur, n_in = init_ap, BPC
                for k in range(DEPTH):
                    dst = buf_tmp if k % 2 == 0 else buf_big
                    for p in range(2):
                        lhsT = w_tile[:, (2 * k + p) * 128:(2 * k + p + 1) * 128]
                        for c0 in range(0, n_in, 512):
                            cw = min(512, n_in - c0)
                            ps = pspool.tile([128, 512], F32, tag="mm")
                            nc.tensor.matmul(
                                ps[:, :cw], lhsT, cur[:, c0:c0 + cw],
                                start=True, stop=True,
                            )
                            nc.vector.tensor_copy(
                                dst[:, p * n_in + c0:p * n_in + c0 + cw],
                                ps[:, :cw],
                            )
                    cur, n_in = dst[:, :2 * n_in], 2 * n_in
                return cur  # (128, 64*BPC)

            Lfin = recursion(aw, ut[:], Ltmp, Lbuf)
            Rfin = recursion(bw, vt[:], Rtmp, Rbuf)

            Lr = Lfin.rearrange("p (j b) -> p j b", b=BPC)
            Rr = Rfin.rearrange("p (j b) -> p j b", b=BPC)
            GRP = 8
            n_groups = BPC // GRP

            # SR_b = R_b^T R_b for all batches (runs on PE while DVE still
            # copies layer outputs; srsb copies go to the scalar engine)
            SRsb = spool.tile([64, BPC * 64], F32, tag="SRsb")
            for g in range(n_groups):
                srg = pspool.tile([64, GRP * 64], F32, tag="srg")
                for i in range(GRP):
                    b = g * GRP + i
                    nc.tensor.matmul(
                        srg[:, i * 64:(i + 1) * 64], Rr[:, :, b], Rr[:, :, b],
                        start=True, stop=True,
                    )
                nc.scalar.copy(
                    out=SRsb[:, g * GRP * 64:(g + 1) * GRP * 64], in_=srg[:]
                )

            # P = [G0 @ L | G1 @ L]  -> (128, (q, j, b))
            NL = 64 * BPC  # 4096
            for q in range(2):
                for c0 in range(0, NL, 512):
                    ps = pspool.tile([128, 512], F32, tag="mm")
                    nc.tensor.matmul(
                        ps[:], gw[:, q * DA:(q + 1) * DA], Lfin[:, c0:c0 + 512],
                        start=True, stop=True,
                    )
                    nc.vector.tensor_copy(Pbuf[:, q * NL + c0:q * NL + c0 + 512], ps[:])

            Pr = Pbuf[:].rearrange("p (q j b) -> p q j b", q=2, b=BPC)
            SRr = SRsb[:].rearrange("p (i j) -> p i j", j=64)

            # Per-batch quadratic forms, 8 batches per group.
            for g in range(n_groups):
                slg = pspool.tile([64, GRP * 128], F32, tag="slg")
                for i in range(GRP):
                    b = g * GRP + i
                    nc.tensor.matmul(
                        slg[:, i * 128:(i + 1) * 128], Lr[:, :, b], Pr[:, :, :, b],
                        start=True, stop=True,
                    )
                slg_r = slg[:].rearrange("p (i q j) -> p i q j", q=2, j=64)
                srsb_r = SRr[:, g * GRP:(g + 1) * GRP, :]
                t0 = gpool.tile([64, GRP * 64], F32, tag="t0")
                t1 = gpool.tile([64, GRP * 64], F32, tag="t1")
                t0_r = t0[:].rearrange("p (i j) -> p i j", j=64)
                t1_r = t1[:].rearrange("p (i j) -> p i j", j=64)
                nc.vector.tensor_mul(t0_r, slg_r[:, :, 0, :], srsb_r)
                nc.vector.tensor_mul(t1_r, slg_r[:, :, 1, :], srsb_r)
                nc.vector.reduce_sum(
                    out=Z[:, g * GRP:(g + 1) * GRP], in_=t0_r,
                    axis=mybir.AxisListType.X,
                )
                nc.vector.reduce_sum(
                    out=Z[:, BPC + g * GRP:BPC + (g + 1) * GRP], in_=t1_r,
                    axis=mybir.AxisListType.X,
                )

            # e[q*BPC + b] = sum over the 64 term-partitions
            zps = pspool.tile([1, 2 * BPC], F32, tag="mm")
            nc.tensor.matmul(zps[:], ones[:], Z[:], start=True, stop=True)
            nc.vector.tensor_copy(esb[:], zps[:])
            nc.sync.dma_start(out=out_d[:, :], in_=esb[:])

    nc.compile()
    return nc


def _get_nc():
    if "nc" not in _nc_cache:
        _nc_cache["nc"] = _build_nc()
    return _nc_cache["nc"]


# ----------------------------------------------------------------------------
# Entry point
# ----------------------------------------------------------------------------

def kernel(X, theta, A, B, D, _trace=False):
    U, V, AT, BT, G = _host_prep(X, theta, A, B, D)
    np_mm = np.float32 if MM_DT == mybir.dt.float32 else mybir.dt.np(MM_DT)
    at = np.ascontiguousarray(AT, dtype=np_mm)
    bt = np.ascontiguousarray(BT, dtype=np_mm)
    g = np.ascontiguousarray(G, dtype=np_mm)
    in_maps = []
    for i in range(N_CORES):
        sl = slice(i * BPC, (i + 1) * BPC)
        in_maps.append(
            {
                "ut": np.ascontiguousarray(U[sl].T, dtype=np_mm),
                "vt": np.ascontiguousarray(V[sl].T, dtype=np_mm),
                "at": at,
                "bt": bt,
                "g": g,
            }
        )
    nc = _get_nc()
    kw = {}
    if _trace:
        import shutil
        import tempfile

        shutil.rmtree("/tmp/vqc_prof", ignore_errors=True)
        os.makedirs("/tmp/vqc_prof", exist_ok=True)
        kw["tmpdir"] = tempfile.mkdtemp(dir="/tmp/vqc_prof")
    res = run_bass_kernel_spmd(nc, in_maps, list(range(N_CORES)), trace=_trace, **kw)
    outs = []
    for i in range(N_CORES):
        e = res.results[i]["out"].reshape(2, BPC).T  # (64, 2)
        outs.append(e)
    full = np.concatenate(outs, axis=0).astype(np.float32)
    if _trace:
        _nc_cache["last_exec_ns"] = res.exec_time_ns
        _nc_cache["last_results"] = res
    return full


# revision 11
# speedup vs baseline: 1.1453x; 1.1453x over previous
"""Trainium2 Bass kernel for nn_ANO_VQC_Model (14-qubit VQC, batch 512).

Math: the circuit's state, viewed as a 128x128 matrix M (rows = qubits 0-6,
cols = qubits 7-13), starts as a real rank-1 outer product u v^T (RY layer on
|+>^14 gives a real product state) and each entangling layer acts as
    M' = A0 M B0^T + A1 M B1^T
(only CNOT(6,7) couples rows and cols; it splits into 2 terms via projectors
on qubit 6).  So the state stays factored: L <- [A0 L | A1 L],
R <- [B0 R | B1 R], M = L R^T with rank <= 64 after 6 layers.  Everything is
real f32.  The two requested expectation values are
    e_q = sum( (L^T G_q L) * (R^T R) ),  G_q = Re(H_q) (x) I_16  (row space).

Sharding: pure data parallel, 64 batch elements per core on 8 cores.
"""

import os
import sys

import numpy as np

for _p in ("/opt/trn_rl_repo", "/root/.axon_site/_ro/trn_rl_repo"):
    if os.path.isdir(_p) and _p not in sys.path:
        sys.path.append(_p)

import concourse.bass as bass
import concourse.mybir as mybir
import concourse.tile as tile
from concourse import bacc
from concourse.bass_utils import run_bass_kernel_spmd

N_CORES = 8
BATCH = 512
BPC = BATCH // N_CORES  # 64
NQ = 14
DEPTH = 6
DA = 128  # row space (qubits 0-6)
DB = 128  # col space (qubits 7-13)

F32 = mybir.dt.float32
# dtype used for the matmul input tensors (weights / L / R / P buffers)
MM_DT = mybir.dt.bfloat16

_nc_cache = {}


# ----------------------------------------------------------------------------
# Host-side preprocessing (input-dependent constant folding)
# ----------------------------------------------------------------------------

def _ry(theta):
    c, s = np.cos(theta / 2), np.sin(theta / 2)
    return np.array([[c, -s], [s, c]], dtype=np.float64)


_CNOT = np.array(
    [[1, 0, 0, 0], [0, 1, 0, 0], [0, 0, 0, 1], [0, 0, 1, 0]], dtype=np.float64
)


def _kron_list(mats):
    out = mats[0]
    for m in mats[1:]:
        out = np.kron(out, m)
    return out


def _cnot_on(n, ctrl):
    mats, q = [], 0
    while q < n:
        if q == ctrl:
            mats.append(_CNOT)
            q += 2
        else:
            mats.append(np.eye(2))
            q += 1
    return _kron_list(mats)


def _layer_mats(theta_k):
    """A0, A1 (row ops) and B0, B1 (col ops) for one entangling layer."""
    C_evenA = _cnot_on(7, 0) @ _cnot_on(7, 2) @ _cnot_on(7, 4)
    C_oddA = _cnot_on(7, 1) @ _cnot_on(7, 3) @ _cnot_on(7, 5)
    R_A = _kron_list([_ry(theta_k[w]) for w in range(7)])
    C_evenB = _cnot_on(7, 1) @ _cnot_on(7, 3) @ _cnot_on(7, 5)
    C_oddB = _cnot_on(7, 0) @ _cnot_on(7, 2) @ _cnot_on(7, 4)
    R_B = _kron_list([_ry(theta_k[7 + w]) for w in range(7)])
    rows = np.arange(DA)
    P0 = np.diag((rows % 2 == 0).astype(np.float64))
    P1 = np.diag((rows % 2 == 1).astype(np.float64))
    S = np.zeros((DB, DB))
    S[: DB // 2, DB // 2:] = np.eye(DB // 2)
    S[DB // 2:, : DB // 2] = np.eye(DB // 2)
    A0 = R_A @ C_oddA @ P0 @ C_evenA
    A1 = R_A @ C_oddA @ P1 @ C_evenA
    B0 = R_B @ C_oddB @ C_evenB
    B1 = R_B @ C_oddB @ S @ C_evenB
    return A0, A1, B0, B1


def _measure_mats(A, B, D):
    """G_q = Re(H_q) expanded to the 128-dim row space, q = 0, 1."""
    NLOC = 8
    rows_t, cols_t = np.tril_indices(NLOC, -1)
    Gs = []
    for q in range(2):
        tri = np.zeros((NLOC, NLOC))
        tri[rows_t, cols_t] = A[q]
        h = tri + np.diag(np.concatenate([D[q][1:], [0.0]]))
        Hr = h + h.T
        if q == 0:
            G = np.kron(Hr, np.eye(16))  # wires 0,1,2 -> row bits 0-2
        else:
            G = np.kron(np.kron(np.eye(2), Hr), np.eye(8))  # wires 1,2,3
        Gs.append(G)
    return np.stack(Gs)


def _host_prep(X, theta, A, B, D):
    X = np.asarray(X, dtype=np.float64)
    theta = np.asarray(theta, dtype=np.float64)
    A = np.asarray(A, dtype=np.float64)
    B = np.asarray(B, dtype=np.float64)
    D = np.asarray(D, dtype=np.float64)
    nb = X.shape[0]
    c, s = np.cos(X / 2), np.sin(X / 2)
    v0 = (c - s) / np.sqrt(2.0)
    v1 = (c + s) / np.sqrt(2.0)

    def kron_side(ws):
        out = np.ones((nb, 1))
        for w in ws:
            pair = np.stack([v0[:, w], v1[:, w]], axis=1)
            out = (out[:, :, None] * pair[:, None, :]).reshape(nb, -1)
        return out

    U = kron_side(range(7))  # (B, 128)
    V = kron_side(range(7, 14))
    AT = np.empty((2 * DEPTH, DA, DA))
    BT = np.empty((2 * DEPTH, DB, DB))
    for k in range(DEPTH):
        A0, A1, B0, B1 = _layer_mats(theta[k])
        AT[2 * k + 0] = A0.T  # lhsT layout: out = lhsT.T @ rhs
        AT[2 * k + 1] = A1.T
        BT[2 * k + 0] = B0.T
        BT[2 * k + 1] = B1.T
    G = _measure_mats(A, B, D)  # (2, 128, 128), symmetric
    return U, V, AT, BT, G


# ----------------------------------------------------------------------------
# Device kernel
# ----------------------------------------------------------------------------

def _build_nc():
    nc = bacc.Bacc("TRN2", target_bir_lowering=False, debug=False)

    ut_d = nc.declare_dram_parameter("ut", [DA, BPC], MM_DT, isOutput=False)
    vt_d = nc.declare_dram_parameter("vt", [DB, BPC], MM_DT, isOutput=False)
    at_d = nc.declare_dram_parameter("at", [2 * DEPTH, DA, DA], MM_DT, isOutput=False)
    bt_d = nc.declare_dram_parameter("bt", [2 * DEPTH, DB, DB], MM_DT, isOutput=False)
    g_d = nc.declare_dram_parameter("g", [2, DA, DA], MM_DT, isOutput=False)
    out_d = nc.declare_dram_parameter("out", [1, 2 * BPC], F32, isOutput=True)

    with tile.TileContext(nc) as tc:
        with (
            tc.tile_pool(name="w", bufs=1) as wpool,
            tc.tile_pool(name="state", bufs=1) as spool,
            tc.tile_pool(name="grp", bufs=2) as gpool,
            tc.tile_pool(name="ps", bufs=2, space="PSUM") as pspool,
        ):
            aw = wpool.tile([DA, 2 * DEPTH * DA], MM_DT, tag="aw")
            bw = wpool.tile([DB, 2 * DEPTH * DB], MM_DT, tag="bw")
            gw = wpool.tile([DA, 2 * DA], MM_DT, tag="gw")
            ut = wpool.tile([DA, BPC], MM_DT, tag="ut")
            vt = wpool.tile([DB, BPC], MM_DT, tag="vt")
            ones = wpool.tile([64, 1], MM_DT, tag="ones")

            # ut/vt first (layer-1 matmuls need them), one layer-pair of
            # weights per DMA, split across the two HWDGE queues (sync+scalar)
            nc.sync.dma_start(out=ut[:], in_=ut_d[:, :])
            nc.scalar.dma_start(out=vt[:], in_=vt_d[:, :])
            for k in range(DEPTH):
                src_a = at_d[2 * k:2 * k + 2].rearrange("i p m -> p i m")
                dst_a = aw[:, k * 256:(k + 1) * 256].rearrange(
                    "p (i m) -> p i m", i=2
                )
                nc.sync.dma_start(out=dst_a, in_=src_a)
                src_b = bt_d[2 * k:2 * k + 2].rearrange("i p m -> p i m")
                dst_b = bw[:, k * 256:(k + 1) * 256].rearrange(
                    "p (i m) -> p i m", i=2
                )
                nc.scalar.dma_start(out=dst_b, in_=src_b)
            nc.sync.dma_start(
                out=gw[:].rearrange("p (q m) -> p q m", q=2),
                in_=g_d[:].rearrange("q p m -> p q m"),
            )
            nc.vector.memset(ones[:], 1.0)

            Ltmp = spool.tile([DA, 32 * BPC], MM_DT, tag="Ltmp")
            Lbuf = spool.tile([DA, 64 * BPC], MM_DT, tag="Lbuf")
            Rtmp = spool.tile([DB, 32 * BPC], MM_DT, tag="Rtmp")
            Rbuf = spool.tile([DB, 64 * BPC], MM_DT, tag="Rbuf")
            Pbuf = spool.tile([DA, 2 * 64 * BPC], MM_DT, tag="Pbuf")
            esb = spool.tile([1, 2 * BPC], F32, tag="esb")

            # Interleaved L/R layer recursion, b-major column layout:
            # col index = b * nj + j  (nj = terms per batch at this level).
            def layer(w_tile, cur, dst, k, n_in):
                nj_in = n_in // BPC
                if n_in <= 256:
                    # both p terms in one PSUM tile, single strided cast out
                    ps = pspool.tile([128, 512], F32, tag="mm")
                    for p in range(2):
                        lhsT = w_tile[:, (2 * k + p) * 128:(2 * k + p + 1) * 128]
                        nc.tensor.matmul(
                            ps[:, p * n_in:(p + 1) * n_in], lhsT, cur[:, :n_in],
                            start=True, stop=True,
                        )
                    src = ps[:, :2 * n_in].rearrange(
                        "pp (t b j) -> pp b t j", t=2, b=BPC
                    )
                    dstv = dst[:, :2 * n_in].rearrange(
                        "pp (b t j) -> pp b t j", t=2, j=nj_in
                    )
                    nc.vector.tensor_copy(dstv, src)
                else:
                    dstv = dst[:, :2 * n_in].rearrange(
                        "pp (b t j) -> pp b t j", t=2, j=nj_in
                    )
                    for p in range(2):
                        lhsT = w_tile[:, (2 * k + p) * 128:(2 * k + p + 1) * 128]
                        for c0 in range(0, n_in, 512):
                            nb = 512 // nj_in
                            b0 = c0 // nj_in
                            ps = pspool.tile([128, 512], F32, tag="mm")
                            nc.tensor.matmul(
                                ps[:], lhsT, cur[:, c0:c0 + 512],
                                start=True, stop=True,
                            )
                            src = ps[:].rearrange("pp (b j) -> pp b j", j=nj_in)
                            nc.vector.tensor_copy(
                                dstv[:, b0:b0 + nb, p, :], src
                            )
                return dst[:, :2 * n_in], 2 * n_in

            curL, nL = ut[:], BPC
            curR, nR = vt[:], BPC
            for k in range(DEPTH):
                dstL = Ltmp if k % 2 == 0 else Lbuf
                dstR = Rtmp if k % 2 == 0 else Rbuf
                curL, nL = layer(aw, curL, dstL, k, nL)
                curR, nR = layer(bw, curR, dstR, k, nR)
            Lfin, Rfin = curL, curR

            GRP = 8
            n_groups = BPC // GRP

            # SR_b = R_b^T R_b for all batches (runs on PE while DVE still
            # copies layer outputs; srsb copies go to the scalar engine)
            SRsb = spool.tile([64, BPC * 64], F32, tag="SRsb")
            for g in range(n_groups):
                srg = pspool.tile([64, GRP * 64], F32, tag="srg")
                for i in range(GRP):
                    b = g * GRP + i
                    rb = Rfin[:, b * 64:(b + 1) * 64]
                    nc.tensor.matmul(
                        srg[:, i * 64:(i + 1) * 64], rb, rb,
                        start=True, stop=True,
                    )
                nc.scalar.copy(
                    out=SRsb[:, g * GRP * 64:(g + 1) * GRP * 64], in_=srg[:]
                )

            # P = [G0 @ L | G1 @ L]  -> (128, (q, b, j))
            NL = 64 * BPC  # 4096
            for q in range(2):
                for c0 in range(0, NL, 512):
                    ps = pspool.tile([128, 512], F32, tag="mm")
                    nc.tensor.matmul(
                        ps[:], gw[:, q * DA:(q + 1) * DA], Lfin[:, c0:c0 + 512],
                        start=True, stop=True,
                    )
                    dst = Pbuf[:, q * NL + c0:q * NL + c0 + 512]
                    if q == 0:
                        nc.vector.tensor_copy(dst, ps[:])
                    else:
                        nc.scalar.copy(out=dst, in_=ps[:])

            Pr = Pbuf[:].rearrange("p (q b j) -> p q b j", q=2, j=64)
            SRr = SRsb[:].rearrange("p (i j) -> p i j", j=64)

            # Per-batch quadratic forms, 8 batches per group.
            for g in range(n_groups):
                slg = pspool.tile([64, GRP * 128], F32, tag="slg")
                for i in range(GRP):
                    b = g * GRP + i
                    nc.tensor.matmul(
                        slg[:, i * 128:(i + 1) * 128],
                        Lfin[:, b * 64:(b + 1) * 64], Pr[:, :, b, :],
                        start=True, stop=True,
                    )
                slg_r = slg[:].rearrange("p (i q j) -> p i q j", q=2, j=64)
                srsb_r = SRr[:, g * GRP:(g + 1) * GRP, :]
                for q in range(2):
                    t = gpool.tile([64, GRP * 64], MM_DT, tag=f"t{q}")
                    t_r = t[:].rearrange("p (i j) -> p i j", j=64)
                    nc.vector.tensor_mul(t_r, slg_r[:, :, q, :], srsb_r)
                    # sum over the 64 term-partitions via ones-matmul,
                    # then the tiny j-reduction on DVE
                    zp = pspool.tile([1, GRP * 64], F32, tag="mm")
                    nc.tensor.matmul(zp[:], ones[:], t[:], start=True, stop=True)
                    nc.vector.reduce_sum(
                        out=esb[0:1, q * BPC + g * GRP:q * BPC + (g + 1) * GRP],
                        in_=zp[:].rearrange("p (i j) -> p i j", j=64),
                        axis=mybir.AxisListType.X,
                    )

            nc.sync.dma_start(out=out_d[:, :], in_=esb[:])

    nc.compile()
    return nc


def _get_nc():
    if "nc" not in _nc_cache:
        _nc_cache["nc"] = _build_nc()
    return _nc_cache["nc"]


# ----------------------------------------------------------------------------
# Entry point
# ----------------------------------------------------------------------------

def kernel(X, theta, A, B, D, _trace=False):
    U, V, AT, BT, G = _host_prep(X, theta, A, B, D)
    np_mm = np.float32 if MM_DT == mybir.dt.float32 else mybir.dt.np(MM_DT)
    at = np.ascontiguousarray(AT, dtype=np_mm)
    bt = np.ascontiguousarray(BT, dtype=np_mm)
    g = np.ascontiguousarray(G, dtype=np_mm)
    in_maps = []
    for i in range(N_CORES):
        sl = slice(i * BPC, (i + 1) * BPC)
        in_maps.append(
            {
                "ut": np.ascontiguousarray(U[sl].T, dtype=np_mm),
                "vt": np.ascontiguousarray(V[sl].T, dtype=np_mm),
                "at": at,
                "bt": bt,
                "g": g,
            }
        )
    nc = _get_nc()
    kw = {}
    if _trace:
        import shutil
        import tempfile

        shutil.rmtree("/tmp/vqc_prof", ignore_errors=True)
        os.makedirs("/tmp/vqc_prof", exist_ok=True)
        kw["tmpdir"] = tempfile.mkdtemp(dir="/tmp/vqc_prof")
    res = run_bass_kernel_spmd(nc, in_maps, list(range(N_CORES)), trace=_trace, **kw)
    outs = []
    for i in range(N_CORES):
        e = res.results[i]["out"].reshape(2, BPC).T  # (64, 2)
        outs.append(e)
    full = np.concatenate(outs, axis=0).astype(np.float32)
    if _trace:
        _nc_cache["last_exec_ns"] = res.exec_time_ns
        _nc_cache["last_results"] = res
    return full


# revision 13
# speedup vs baseline: 1.5837x; 1.3828x over previous
"""Trainium2 Bass kernel for nn_ANO_VQC_Model (14-qubit VQC, batch 512).

Math: the circuit's state, viewed as a 128x128 matrix M (rows = qubits 0-6,
cols = qubits 7-13), starts as a real rank-1 outer product u v^T (RY layer on
|+>^14 gives a real product state) and each entangling layer acts as
    M' = A0 M B0^T + A1 M B1^T
(only CNOT(6,7) couples rows and cols; it splits into 2 terms via projectors
on qubit 6).  So the state stays factored: L <- [A0 L | A1 L],
R <- [B0 R | B1 R], M = L R^T with rank <= 64 after 6 layers.  Everything is
real f32.  The two requested expectation values are
    e_q = sum( (L^T G_q L) * (R^T R) ),  G_q = Re(H_q) (x) I  (row space).

The row basis is rotated (host-side, folded into the last layer's A matrices)
so that G_0 becomes diagonal: its L^T G_0 L then needs only a per-partition
scale of L instead of a matmul; G_1 is expressed in the same basis.

Sharding: pure data parallel, 64 batch elements per core on 8 cores.
"""

import os
import sys

import numpy as np

for _p in ("/opt/trn_rl_repo", "/root/.axon_site/_ro/trn_rl_repo"):
    if os.path.isdir(_p) and _p not in sys.path:
        sys.path.append(_p)

import concourse.bass as bass
import concourse.mybir as mybir
import concourse.tile as tile
from concourse import bacc
from concourse.bass_utils import run_bass_kernel_spmd

N_CORES = 8
BATCH = 512
BPC = BATCH // N_CORES  # 64
NQ = 14
DEPTH = 6
DA = 128
DB = 128

F32 = mybir.dt.float32
MM_DT = mybir.dt.bfloat16  # matmul input dtype

_nc_cache = {}


# ----------------------------------------------------------------------------
# Host-side preprocessing (input-dependent constant folding)
# ----------------------------------------------------------------------------

def _ry(theta):
    c, s = np.cos(theta / 2), np.sin(theta / 2)
    return np.array([[c, -s], [s, c]], dtype=np.float64)


_CNOT = np.array(
    [[1, 0, 0, 0], [0, 1, 0, 0], [0, 0, 0, 1], [0, 0, 1, 0]], dtype=np.float64
)


def _kron_list(mats):
    out = mats[0]
    for m in mats[1:]:
        out = np.kron(out, m)
    return out


def _cnot_on(n, ctrl):
    mats, q = [], 0
    while q < n:
        if q == ctrl:
            mats.append(_CNOT)
            q += 2
        else:
            mats.append(np.eye(2))
            q += 1
    return _kron_list(mats)


def _layer_mats(theta_k):
    C_evenA = _cnot_on(7, 0) @ _cnot_on(7, 2) @ _cnot_on(7, 4)
    C_oddA = _cnot_on(7, 1) @ _cnot_on(7, 3) @ _cnot_on(7, 5)
    R_A = _kron_list([_ry(theta_k[w]) for w in range(7)])
    C_evenB = _cnot_on(7, 1) @ _cnot_on(7, 3) @ _cnot_on(7, 5)
    C_oddB = _cnot_on(7, 0) @ _cnot_on(7, 2) @ _cnot_on(7, 4)
    R_B = _kron_list([_ry(theta_k[7 + w]) for w in range(7)])
    rows = np.arange(DA)
    P0 = np.diag((rows % 2 == 0).astype(np.float64))
    P1 = np.diag((rows % 2 == 1).astype(np.float64))
    S = np.zeros((DB, DB))
    S[: DB // 2, DB // 2:] = np.eye(DB // 2)
    S[DB // 2:, : DB // 2] = np.eye(DB // 2)
    A0 = R_A @ C_oddA @ P0 @ C_evenA
    A1 = R_A @ C_oddA @ P1 @ C_evenA
    B0 = R_B @ C_oddB @ C_evenB
    B1 = R_B @ C_oddB @ S @ C_evenB
    return A0, A1, B0, B1


def _measure_mats(A, B, D):
    """Re(H_q) expanded to the 128-dim row space, q = 0, 1."""
    NLOC = 8
    rows_t, cols_t = np.tril_indices(NLOC, -1)
    Gs = []
    for q in range(2):
        tri = np.zeros((NLOC, NLOC))
        tri[rows_t, cols_t] = A[q]
        h = tri + np.diag(np.concatenate([D[q][1:], [0.0]]))
        Hr = h + h.T
        if q == 0:
            G = np.kron(Hr, np.eye(16))  # wires 0,1,2 -> row bits 0-2
        else:
            G = np.kron(np.kron(np.eye(2), Hr), np.eye(8))  # wires 1,2,3
        Gs.append(G)
    return Gs


def _host_prep(X, theta, A, B, D):
    X = np.asarray(X, dtype=np.float64)
    theta = np.asarray(theta, dtype=np.float64)
    A = np.asarray(A, dtype=np.float64)
    B = np.asarray(B, dtype=np.float64)
    D = np.asarray(D, dtype=np.float64)
    nb = X.shape[0]
    c, s = np.cos(X / 2), np.sin(X / 2)
    v0 = (c - s) / np.sqrt(2.0)
    v1 = (c + s) / np.sqrt(2.0)

    def kron_side(ws):
        out = np.ones((nb, 1))
        for w in ws:
            pair = np.stack([v0[:, w], v1[:, w]], axis=1)
            out = (out[:, :, None] * pair[:, None, :]).reshape(nb, -1)
        return out

    U = kron_side(range(7))  # (B, 128)
    V = kron_side(range(7, 14))

    G0, G1 = _measure_mats(A, B, D)
    # rotate the row basis so G0 is diagonal: G0 = Hr0 (x) I16,
    # Hr0 = W L W^T  ->  (W (x) I)^T G0 (W (x) I) = diag(repeat(lam, 16))
    NLOC = 8
    rows_t, cols_t = np.tril_indices(NLOC, -1)
    tri = np.zeros((NLOC, NLOC))
    tri[rows_t, cols_t] = A[0]
    h = tri + np.diag(np.concatenate([D[0][1:], [0.0]]))
    Hr0 = h + h.T
    lam8, W0 = np.linalg.eigh(Hr0)
    Wk = np.kron(W0, np.eye(16))  # orthogonal, 128x128
    lam = np.repeat(lam8, 16)  # (128,)
    G1r = Wk.T @ G1 @ Wk  # G1 in the rotated basis (symmetric)

    AT = np.empty((2 * DEPTH, DA, DA))
    BT = np.empty((2 * DEPTH, DB, DB))
    for k in range(DEPTH):
        A0, A1, B0, B1 = _layer_mats(theta[k])
        if k == DEPTH - 1:
            A0 = Wk.T @ A0  # fold the rotation into the last layer
            A1 = Wk.T @ A1
        AT[2 * k + 0] = A0.T  # lhsT layout: out = lhsT.T @ rhs
        AT[2 * k + 1] = A1.T
        BT[2 * k + 0] = B0.T
        BT[2 * k + 1] = B1.T
    # pack partition-major for contiguous DMA: (128, 12*128)
    at_pack = np.ascontiguousarray(AT.transpose(1, 0, 2).reshape(DA, -1))
    bt_pack = np.ascontiguousarray(BT.transpose(1, 0, 2).reshape(DB, -1))
    return U, V, at_pack, bt_pack, G1r, lam


# ----------------------------------------------------------------------------
# Device kernel
# ----------------------------------------------------------------------------

def _build_nc():
    nc = bacc.Bacc("TRN2", target_bir_lowering=False, debug=False)

    ut_d = nc.declare_dram_parameter("ut", [DA, BPC], MM_DT, isOutput=False)
    vt_d = nc.declare_dram_parameter("vt", [DB, BPC], MM_DT, isOutput=False)
    at_d = nc.declare_dram_parameter("at", [DA, 2 * DEPTH * DA], MM_DT, isOutput=False)
    bt_d = nc.declare_dram_parameter("bt", [DB, 2 * DEPTH * DB], MM_DT, isOutput=False)
    g_d = nc.declare_dram_parameter("g", [DA, DA], MM_DT, isOutput=False)
    lam_d = nc.declare_dram_parameter("lam", [DA, 1], F32, isOutput=False)
    out_d = nc.declare_dram_parameter("out", [2, BPC], F32, isOutput=True)

    cast_cnt = [0]

    with tile.TileContext(nc) as tc:
        with (
            tc.tile_pool(name="w", bufs=1) as wpool,
            tc.tile_pool(name="state", bufs=1) as spool,
            tc.tile_pool(name="grp", bufs=2) as gpool,
            tc.tile_pool(name="ps", bufs=2, space="PSUM") as pspool,
        ):
            aw = wpool.tile([DA, 2 * DEPTH * DA], MM_DT, tag="aw")
            bw = wpool.tile([DB, 2 * DEPTH * DB], MM_DT, tag="bw")
            gw = wpool.tile([DA, DA], MM_DT, tag="gw")
            ut = wpool.tile([DA, BPC], MM_DT, tag="ut")
            vt = wpool.tile([DB, BPC], MM_DT, tag="vt")
            lam = wpool.tile([DA, 1], F32, tag="lam")
            ones2 = wpool.tile([128, 2], MM_DT, tag="ones2")

            nc.sync.dma_start(out=ut[:], in_=ut_d[:, :])
            nc.scalar.dma_start(out=vt[:], in_=vt_d[:, :])
            nc.sync.dma_start(out=aw[:], in_=at_d[:, :])
            nc.scalar.dma_start(out=bw[:], in_=bt_d[:, :])
            nc.sync.dma_start(out=gw[:], in_=g_d[:, :])
            nc.scalar.dma_start(out=lam[:], in_=lam_d[:, :])
            nc.vector.memset(ones2[:], 0.0)
            nc.vector.memset(ones2[0:64, 0:1], 1.0)
            nc.vector.memset(ones2[64:128, 1:2], 1.0)

            Ltmp = spool.tile([DA, 32 * BPC], MM_DT, tag="Ltmp")
            Lbuf = spool.tile([DA, 64 * BPC], MM_DT, tag="Lbuf")
            Rtmp = spool.tile([DB, 32 * BPC], MM_DT, tag="Rtmp")
            Rbuf = spool.tile([DB, 64 * BPC], MM_DT, tag="Rbuf")
            Pbuf = spool.tile([DA, 2 * 64 * BPC], MM_DT, tag="Pbuf")
            SRsb = spool.tile([128, 32 * 64], F32, tag="SRsb")
            esb = spool.tile([2, BPC], F32, tag="esb")

            def cast_out(dst_ap, src_ap):
                # alternate PSUM->SBUF copies between DVE and ACT
                if cast_cnt[0] % 2 == 0:
                    nc.vector.tensor_copy(dst_ap, src_ap)
                else:
                    nc.scalar.copy(out=dst_ap, in_=src_ap)
                cast_cnt[0] += 1

            # ---- interleaved L/R layer recursion, b-major columns ----------
            def layer(w_tile, cur, dst, k, n_in):
                nj_in = n_in // BPC
                dstv = dst[:, :2 * n_in].rearrange(
                    "pp (b t j) -> pp b t j", t=2, j=nj_in
                )
                if n_in <= 256:
                    ps = pspool.tile([128, 1024], F32, tag="mm2")
                    for p in range(2):
                        lhsT = w_tile[:, (2 * k + p) * 128:(2 * k + p + 1) * 128]
                        nc.tensor.matmul(
                            ps[:, p * n_in:(p + 1) * n_in], lhsT, cur[:, :n_in],
                            start=True, stop=True,
                        )
                    src = ps[:, :2 * n_in].rearrange(
                        "pp (t b j) -> pp b t j", t=2, b=BPC
                    )
                    cast_out(dstv, src)
                elif n_in == 512:
                    # both p streams (512 each) into one 1024 psum tile
                    ps = pspool.tile([128, 1024], F32, tag="mm2")
                    for p in range(2):
                        lhsT = w_tile[:, (2 * k + p) * 128:(2 * k + p + 1) * 128]
                        nc.tensor.matmul(
                            ps[:, p * 512:(p + 1) * 512], lhsT, cur[:, :512],
                            start=True, stop=True,
                        )
                    src = ps[:].rearrange("pp (t b j) -> pp b t j", t=2, b=BPC)
                    cast_out(dstv, src)
                else:
                    # n_in >= 1024: per p, 1024-col psum units
                    nb_unit = 1024 // nj_in
                    for p in range(2):
                        lhsT = w_tile[:, (2 * k + p) * 128:(2 * k + p + 1) * 128]
                        for c0 in range(0, n_in, 1024):
                            ps = pspool.tile([128, 1024], F32, tag="mm2")
                            nc.tensor.matmul(
                                ps[:, 0:512], lhsT, cur[:, c0:c0 + 512],
                                start=True, stop=True,
                            )
                            nc.tensor.matmul(
                                ps[:, 512:1024], lhsT, cur[:, c0 + 512:c0 + 1024],
                                start=True, stop=True,
                            )
                            b0 = c0 // nj_in
                            src = ps[:].rearrange("pp (b j) -> pp b j", j=nj_in)
                            cast_out(dstv[:, b0:b0 + nb_unit, p, :], src)
                return dst[:, :2 * n_in], 2 * n_in

            curL, nL = ut[:], BPC
            curR, nR = vt[:], BPC
            for k in range(DEPTH):
                dstL = Ltmp if k % 2 == 0 else Lbuf
                dstR = Rtmp if k % 2 == 0 else Rbuf
                curL, nL = layer(aw, curL, dstL, k, nL)
                curR, nR = layer(bw, curR, dstR, k, nR)
            Lfin, Rfin = curL, curR

            GRP = 8
            n_groups = BPC // GRP
            n_slots = GRP // 2

            # ---- SR_b = R_b^T R_b, two batches per PE pass via col tiling --
            for g in range(n_groups):
                srg = pspool.tile([128, n_slots * 64], F32, tag="srg")
                for s_ in range(n_slots):
                    b0 = g * GRP + 2 * s_
                    r0 = Rfin[:, b0 * 64:(b0 + 1) * 64]
                    r1 = Rfin[:, (b0 + 1) * 64:(b0 + 2) * 64]
                    nc.tensor.matmul(
                        srg[0:64, s_ * 64:(s_ + 1) * 64], r0, r0,
                        start=True, stop=True, tile_position=(0, 0),
                    )
                    nc.tensor.matmul(
                        srg[64:128, s_ * 64:(s_ + 1) * 64], r1, r1,
                        start=True, stop=True, tile_position=(0, 64),
                    )
                nc.scalar.copy(
                    out=SRsb[:, g * n_slots * 64:(g + 1) * n_slots * 64],
                    in_=srg[:],
                )

            # ---- P = [diag(lam) L | G1' L]  -> (128, (q, b, j)) ------------
            NL = 64 * BPC  # 4096
            for c0 in range(0, NL, 2048):
                nc.vector.tensor_scalar_mul(
                    Pbuf[:, c0:c0 + 2048], Lfin[:, c0:c0 + 2048], lam[:]
                )
            for c0 in range(0, NL, 1024):
                ps = pspool.tile([128, 1024], F32, tag="mm2")
                nc.tensor.matmul(
                    ps[:, 0:512], gw[:], Lfin[:, c0:c0 + 512],
                    start=True, stop=True,
                )
                nc.tensor.matmul(
                    ps[:, 512:1024], gw[:], Lfin[:, c0 + 512:c0 + 1024],
                    start=True, stop=True,
                )
                cast_out(Pbuf[:, NL + c0:NL + c0 + 1024], ps[:])

            Pr = Pbuf[:].rearrange("p (q b j) -> p q b j", q=2, j=64)
            SRr = SRsb[:].rearrange("p (s j) -> p s j", j=64)

            # ---- per-batch quadratic forms, 8 batches (4 slots) per group --
            for g in range(n_groups):
                slg = pspool.tile([128, n_slots * 128], F32, tag="slg")
                for s_ in range(n_slots):
                    b0 = g * GRP + 2 * s_
                    nc.tensor.matmul(
                        slg[0:64, s_ * 128:(s_ + 1) * 128],
                        Lfin[:, b0 * 64:(b0 + 1) * 64], Pr[:, :, b0, :],
                        start=True, stop=True, tile_position=(0, 0),
                    )
                    nc.tensor.matmul(
                        slg[64:128, s_ * 128:(s_ + 1) * 128],
                        Lfin[:, (b0 + 1) * 64:(b0 + 2) * 64], Pr[:, :, b0 + 1, :],
                        start=True, stop=True, tile_position=(0, 64),
                    )
                slg_r = slg[:].rearrange("p (s q j) -> p s q j", q=2, j=64)
                srsb_r = SRr[:, g * n_slots:(g + 1) * n_slots, :]
                for q in range(2):
                    t = gpool.tile([128, n_slots * 64], MM_DT, tag=f"t{q}")
                    t_r = t[:].rearrange("p (s j) -> p s j", j=64)
                    nc.vector.tensor_mul(t_r, slg_r[:, :, q, :], srsb_r)
                    zp = pspool.tile([2, n_slots * 64], F32, tag="srg")
                    nc.tensor.matmul(zp[:], ones2[:], t[:], start=True, stop=True)
                    nc.vector.reduce_sum(
                        out=esb[0:2, q * 32 + g * n_slots:q * 32 + (g + 1) * n_slots],
                        in_=zp[:].rearrange("p (s j) -> p s j", j=64),
                        axis=mybir.AxisListType.X,
                    )

            nc.sync.dma_start(out=out_d[:, :], in_=esb[:])

    nc.compile()
    return nc


def _get_nc():
    if "nc" not in _nc_cache:
        _nc_cache["nc"] = _build_nc()
    return _nc_cache["nc"]


# ----------------------------------------------------------------------------
# Entry point
# ----------------------------------------------------------------------------

def _decode_out(raw):
    """raw (2, 64): [parity, q*32 + g*4 + slot] -> (64, 2) e[b, q]."""
    e = np.empty((BPC, 2), dtype=np.float32)
    idx = np.arange(32)
    g, s_ = idx // 4, idx % 4
    b_even = g * 8 + s_ * 2
    for q in range(2):
        e[b_even, q] = raw[0, q * 32 + idx]
        e[b_even + 1, q] = raw[1, q * 32 + idx]
    return e


def kernel(X, theta, A, B, D, _trace=False):
    U, V, at_pack, bt_pack, G1r, lam = _host_prep(X, theta, A, B, D)
    np_mm = mybir.dt.np(MM_DT)
    at = np.ascontiguousarray(at_pack, dtype=np_mm)
    bt = np.ascontiguousarray(bt_pack, dtype=np_mm)
    g = np.ascontiguousarray(G1r, dtype=np_mm)
    lam_a = np.ascontiguousarray(lam.reshape(DA, 1), dtype=np.float32)
    in_maps = []
    for i in range(N_CORES):
        sl = slice(i * BPC, (i + 1) * BPC)
        in_maps.append(
            {
                "ut": np.ascontiguousarray(U[sl].T, dtype=np_mm),
                "vt": np.ascontiguousarray(V[sl].T, dtype=np_mm),
                "at": at,
                "bt": bt,
                "g": g,
                "lam": lam_a,
            }
        )
    nc = _get_nc()
    kw = {}
    if _trace:
        import shutil
        import tempfile

        shutil.rmtree("/tmp/vqc_prof", ignore_errors=True)
        os.makedirs("/tmp/vqc_prof", exist_ok=True)
        kw["tmpdir"] = tempfile.mkdtemp(dir="/tmp/vqc_prof")
    res = run_bass_kernel_spmd(nc, in_maps, list(range(N_CORES)), trace=_trace, **kw)
    outs = [_decode_out(res.results[i]["out"]) for i in range(N_CORES)]
    full = np.concatenate(outs, axis=0).astype(np.float32)
    if _trace:
        _nc_cache["last_exec_ns"] = res.exec_time_ns
        _nc_cache["last_results"] = res
    return full


# revision 19
# speedup vs baseline: 1.7977x; 1.1351x over previous
"""Trainium2 Bass kernel for nn_ANO_VQC_Model (14-qubit VQC, batch 512).

Math: the circuit's state, viewed as a 128x128 matrix M (rows = qubits 0-6,
cols = qubits 7-13), starts as a real rank-1 outer product u v^T (RY layer on
|+>^14 gives a real product state) and each entangling layer acts as
    M' = A0 M B0^T + A1 M B1^T
(only CNOT(6,7) couples rows and cols; it splits into 2 terms via projectors
on qubit 6).  So the state stays factored: L <- [A0 L | A1 L],
R <- [B0 R | B1 R], M = L R^T with rank <= 64 after 6 layers.  Everything is
real f32.  The two requested expectation values are
    e_q = sum( (L^T G_q L) * (R^T R) ),  G_q = Re(H_q) (x) I  (row space).

The row basis is rotated (host-side, folded into the last layer's A matrices)
so that G_0 becomes diagonal: its L^T G_0 L then needs only a per-partition
scale of L instead of a matmul; G_1 is expressed in the same basis.

Sharding: pure data parallel, 64 batch elements per core on 8 cores.
"""

import os
import sys

import numpy as np

for _p in ("/opt/trn_rl_repo", "/root/.axon_site/_ro/trn_rl_repo"):
    if os.path.isdir(_p) and _p not in sys.path:
        sys.path.append(_p)

import concourse.bass as bass
import concourse.mybir as mybir
import concourse.tile as tile
from concourse import bacc
from concourse.bass_utils import run_bass_kernel_spmd

N_CORES = 8
BATCH = 512
BPC = BATCH // N_CORES  # 64
NQ = 14
DEPTH = 6
DA = 128
DB = 128

F32 = mybir.dt.float32
MM_DT = mybir.dt.bfloat16  # matmul input dtype

_nc_cache = {}


# ----------------------------------------------------------------------------
# Host-side preprocessing (input-dependent constant folding)
# ----------------------------------------------------------------------------

def _ry(theta):
    c, s = np.cos(theta / 2), np.sin(theta / 2)
    return np.array([[c, -s], [s, c]], dtype=np.float64)


_CNOT = np.array(
    [[1, 0, 0, 0], [0, 1, 0, 0], [0, 0, 0, 1], [0, 0, 1, 0]], dtype=np.float64
)


def _kron_list(mats):
    out = mats[0]
    for m in mats[1:]:
        out = np.kron(out, m)
    return out


def _cnot_on(n, ctrl):
    mats, q = [], 0
    while q < n:
        if q == ctrl:
            mats.append(_CNOT)
            q += 2
        else:
            mats.append(np.eye(2))
            q += 1
    return _kron_list(mats)


def _layer_mats(theta_k):
    C_evenA = _cnot_on(7, 0) @ _cnot_on(7, 2) @ _cnot_on(7, 4)
    C_oddA = _cnot_on(7, 1) @ _cnot_on(7, 3) @ _cnot_on(7, 5)
    R_A = _kron_list([_ry(theta_k[w]) for w in range(7)])
    C_evenB = _cnot_on(7, 1) @ _cnot_on(7, 3) @ _cnot_on(7, 5)
    C_oddB = _cnot_on(7, 0) @ _cnot_on(7, 2) @ _cnot_on(7, 4)
    R_B = _kron_list([_ry(theta_k[7 + w]) for w in range(7)])
    rows = np.arange(DA)
    P0 = np.diag((rows % 2 == 0).astype(np.float64))
    P1 = np.diag((rows % 2 == 1).astype(np.float64))
    S = np.zeros((DB, DB))
    S[: DB // 2, DB // 2:] = np.eye(DB // 2)
    S[DB // 2:, : DB // 2] = np.eye(DB // 2)
    A0 = R_A @ C_oddA @ P0 @ C_evenA
    A1 = R_A @ C_oddA @ P1 @ C_evenA
    B0 = R_B @ C_oddB @ C_evenB
    B1 = R_B @ C_oddB @ S @ C_evenB
    return A0, A1, B0, B1


def _measure_mats(A, B, D):
    """Re(H_q) expanded to the 128-dim row space, q = 0, 1."""
    NLOC = 8
    rows_t, cols_t = np.tril_indices(NLOC, -1)
    Gs = []
    for q in range(2):
        tri = np.zeros((NLOC, NLOC))
        tri[rows_t, cols_t] = A[q]
        h = tri + np.diag(np.concatenate([D[q][1:], [0.0]]))
        Hr = h + h.T
        if q == 0:
            G = np.kron(Hr, np.eye(16))  # wires 0,1,2 -> row bits 0-2
        else:
            G = np.kron(np.kron(np.eye(2), Hr), np.eye(8))  # wires 1,2,3
        Gs.append(G)
    return Gs


def _host_prep(X, theta, A, B, D):
    X = np.asarray(X, dtype=np.float64)
    theta = np.asarray(theta, dtype=np.float64)
    A = np.asarray(A, dtype=np.float64)
    B = np.asarray(B, dtype=np.float64)
    D = np.asarray(D, dtype=np.float64)
    nb = X.shape[0]
    c, s = np.cos(X / 2), np.sin(X / 2)
    v0 = (c - s) / np.sqrt(2.0)
    v1 = (c + s) / np.sqrt(2.0)

    def kron_side(ws):
        out = np.ones((nb, 1))
        for w in ws:
            pair = np.stack([v0[:, w], v1[:, w]], axis=1)
            out = (out[:, :, None] * pair[:, None, :]).reshape(nb, -1)
        return out

    U = kron_side(range(7))  # (B, 128)
    V = kron_side(range(7, 14))

    G0, G1 = _measure_mats(A, B, D)
    # rotate the row basis so G0 is diagonal: G0 = Hr0 (x) I16,
    # Hr0 = W L W^T  ->  (W (x) I)^T G0 (W (x) I) = diag(repeat(lam, 16))
    NLOC = 8
    rows_t, cols_t = np.tril_indices(NLOC, -1)
    tri = np.zeros((NLOC, NLOC))
    tri[rows_t, cols_t] = A[0]
    h = tri + np.diag(np.concatenate([D[0][1:], [0.0]]))
    Hr0 = h + h.T
    lam8, W0 = np.linalg.eigh(Hr0)
    Wk = np.kron(W0, np.eye(16))  # orthogonal, 128x128
    lam = np.repeat(lam8, 16)  # (128,)
    G1r = Wk.T @ G1 @ Wk  # G1 in the rotated basis (symmetric)

    AT = np.empty((2 * DEPTH, DA, DA))
    BT = np.empty((2 * DEPTH, DB, DB))
    for k in range(DEPTH):
        A0, A1, B0, B1 = _layer_mats(theta[k])
        if k == DEPTH - 1:
            A0 = Wk.T @ A0  # fold the rotation into the last layer
            A1 = Wk.T @ A1
        AT[2 * k + 0] = A0.T  # lhsT layout: out = lhsT.T @ rhs
        AT[2 * k + 1] = A1.T
        BT[2 * k + 0] = B0.T
        BT[2 * k + 1] = B1.T
    # pack per-layer, partition-major for contiguous DMA: (6, 128, 256)
    at_pack = np.ascontiguousarray(
        AT.reshape(DEPTH, 2, DA, DA).transpose(0, 2, 1, 3).reshape(DEPTH, DA, 2 * DA)
    )
    bt_pack = np.ascontiguousarray(
        BT.reshape(DEPTH, 2, DB, DB).transpose(0, 2, 1, 3).reshape(DEPTH, DB, 2 * DB)
    )
    return U, V, at_pack, bt_pack, G1r, lam


# ----------------------------------------------------------------------------
# Device kernel
# ----------------------------------------------------------------------------

def _build_nc():
    nc = bacc.Bacc("TRN2", target_bir_lowering=False, debug=False)

    ut_d = nc.declare_dram_parameter("ut", [DA, BPC], MM_DT, isOutput=False)
    vt_d = nc.declare_dram_parameter("vt", [DB, BPC], MM_DT, isOutput=False)
    at_d = nc.declare_dram_parameter("at", [DEPTH, DA, 2 * DA], MM_DT, isOutput=False)
    bt_d = nc.declare_dram_parameter("bt", [DEPTH, DB, 2 * DB], MM_DT, isOutput=False)
    g_d = nc.declare_dram_parameter("g", [DA, DA], MM_DT, isOutput=False)
    lam_d = nc.declare_dram_parameter("lam", [DA, 1], F32, isOutput=False)
    out_d = nc.declare_dram_parameter("out", [2, BPC], F32, isOutput=True)

    cast_cnt = [0]

    with tile.TileContext(nc) as tc:
        with (
            tc.tile_pool(name="w", bufs=1) as wpool,
            tc.tile_pool(name="state", bufs=1) as spool,
            tc.tile_pool(name="grp", bufs=2) as gpool,
            tc.tile_pool(name="ps", bufs=2, space="PSUM") as pspool,
            tc.tile_pool(name="ps1", bufs=1, space="PSUM") as pspool1,
        ):
            aw = wpool.tile([DA, 2 * DEPTH * DA], MM_DT, tag="aw")
            bw = wpool.tile([DB, 2 * DEPTH * DB], MM_DT, tag="bw")
            gw = wpool.tile([DA, DA], MM_DT, tag="gw")
            ut = wpool.tile([DA, BPC], MM_DT, tag="ut")
            vt = wpool.tile([DB, BPC], MM_DT, tag="vt")
            lam = wpool.tile([DA, 1], F32, tag="lam")
            ones2 = wpool.tile([128, 2], MM_DT, tag="ones2")

            nc.sync.dma_start(out=ut[:], in_=ut_d[:, :])
            nc.scalar.dma_start(out=vt[:], in_=vt_d[:, :])
            for k in range(DEPTH):
                nc.sync.dma_start(out=aw[:, k * 256:(k + 1) * 256], in_=at_d[k])
                nc.scalar.dma_start(out=bw[:, k * 256:(k + 1) * 256], in_=bt_d[k])
            nc.sync.dma_start(out=gw[:], in_=g_d[:, :])
            nc.scalar.dma_start(out=lam[:], in_=lam_d[:, :])
            nc.vector.memset(ones2[:], 0.0)
            nc.vector.memset(ones2[0:64, 0:1], 1.0)
            nc.vector.memset(ones2[64:128, 1:2], 1.0)

            Ltmp = spool.tile([DA, 32 * BPC], MM_DT, tag="Ltmp")
            Lbuf = spool.tile([DA, 64 * BPC], MM_DT, tag="Lbuf")
            Rtmp = spool.tile([DB, 32 * BPC], MM_DT, tag="Rtmp")
            Rbuf = spool.tile([DB, 64 * BPC], MM_DT, tag="Rbuf")
            Pbuf = spool.tile([DA, 2 * 64 * BPC], MM_DT, tag="Pbuf")
            SRsb = spool.tile([128, 32 * 64], F32, tag="SRsb")
            esb = spool.tile([2, BPC], F32, tag="esb")

            def cast_out(dst_ap, src_ap):
                # alternate PSUM->SBUF copies between DVE and ACT
                if cast_cnt[0] % 2 == 0:
                    nc.vector.tensor_copy(dst_ap, src_ap)
                else:
                    nc.scalar.copy(out=dst_ap, in_=src_ap)
                cast_cnt[0] += 1

            # ---- interleaved L/R layer recursion, b-major columns ----------
            def layer(w_tile, cur, dst, k, n_in):
                nj_in = n_in // BPC
                dstv = dst[:, :2 * n_in].rearrange(
                    "pp (b t j) -> pp b t j", t=2, j=nj_in
                )
                if n_in <= 256:
                    ps = pspool.tile([128, 1024], F32, tag="mm2")
                    for p in range(2):
                        lhsT = w_tile[:, (2 * k + p) * 128:(2 * k + p + 1) * 128]
                        nc.tensor.matmul(
                            ps[:, p * n_in:(p + 1) * n_in], lhsT, cur[:, :n_in],
                            start=True, stop=True,
                        )
                    src = ps[:, :2 * n_in].rearrange(
                        "pp (t b j) -> pp b t j", t=2, b=BPC
                    )
                    cast_out(dstv, src)
                elif n_in == 512:
                    # both p streams (512 each) into one 1024 psum tile
                    ps = pspool.tile([128, 1024], F32, tag="mm2")
                    for p in range(2):
                        lhsT = w_tile[:, (2 * k + p) * 128:(2 * k + p + 1) * 128]
                        nc.tensor.matmul(
                            ps[:, p * 512:(p + 1) * 512], lhsT, cur[:, :512],
                            start=True, stop=True,
                        )
                    src = ps[:].rearrange("pp (t b j) -> pp b t j", t=2, b=BPC)
                    cast_out(dstv, src)
                else:
                    # n_in >= 1024: per p, 1024-col psum units
                    nb_unit = 1024 // nj_in
                    for p in range(2):
                        lhsT = w_tile[:, (2 * k + p) * 128:(2 * k + p + 1) * 128]
                        for c0 in range(0, n_in, 1024):
                            ps = pspool.tile([128, 1024], F32, tag="mm2")
                            nc.tensor.matmul(
                                ps[:, 0:512], lhsT, cur[:, c0:c0 + 512],
                                start=True, stop=True,
                            )
                            nc.tensor.matmul(
                                ps[:, 512:1024], lhsT, cur[:, c0 + 512:c0 + 1024],
                                start=True, stop=True,
                            )
                            b0 = c0 // nj_in
                            src = ps[:].rearrange("pp (b j) -> pp b j", j=nj_in)
                            cast_out(dstv[:, b0:b0 + nb_unit, p, :], src)
                return dst[:, :2 * n_in], 2 * n_in

            curL, nL = ut[:], BPC
            curR, nR = vt[:], BPC
            for k in range(DEPTH):
                dstL = Ltmp if k % 2 == 0 else Lbuf
                dstR = Rtmp if k % 2 == 0 else Rbuf
                curL, nL = layer(aw, curL, dstL, k, nL)
                curR, nR = layer(bw, curR, dstR, k, nR)
            Lfin, Rfin = curL, curR

            GRP = 8
            n_groups = BPC // GRP
            n_slots = GRP // 2

            # ---- SR_b = R_b^T R_b, two batches per PE pass via col tiling --
            for g in range(n_groups):
                srg = pspool.tile([128, n_slots * 128], F32, tag="fin")
                for s_ in range(n_slots):
                    b0 = g * GRP + 2 * s_
                    r0 = Rfin[:, b0 * 64:(b0 + 1) * 64]
                    r1 = Rfin[:, (b0 + 1) * 64:(b0 + 2) * 64]
                    nc.tensor.matmul(
                        srg[0:64, s_ * 64:(s_ + 1) * 64], r0, r0,
                        start=True, stop=True, tile_position=(0, 0),
                    )
                    nc.tensor.matmul(
                        srg[64:128, s_ * 64:(s_ + 1) * 64], r1, r1,
                        start=True, stop=True, tile_position=(0, 64),
                    )
                nc.scalar.copy(
                    out=SRsb[:, g * n_slots * 64:(g + 1) * n_slots * 64],
                    in_=srg[:, :n_slots * 64],
                )

            # ---- P = [diag(lam) L | G1' L]  -> (128, (q, b, j)) ------------
            NL = 64 * BPC  # 4096
            for c0 in range(0, NL, 2048):
                nc.vector.tensor_scalar_mul(
                    Pbuf[:, c0:c0 + 2048], Lfin[:, c0:c0 + 2048], lam[:]
                )
            for c0 in range(0, NL, 1024):
                ps = pspool.tile([128, 1024], F32, tag="mm2")
                nc.tensor.matmul(
                    ps[:, 0:512], gw[:], Lfin[:, c0:c0 + 512],
                    start=True, stop=True,
                )
                nc.tensor.matmul(
                    ps[:, 512:1024], gw[:], Lfin[:, c0 + 512:c0 + 1024],
                    start=True, stop=True,
                )
                cast_out(Pbuf[:, NL + c0:NL + c0 + 1024], ps[:])

            Pr = Pbuf[:].rearrange("p (q b j) -> p q b j", q=2, j=64)
            SRr = SRsb[:].rearrange("p (s j) -> p s j", j=64)

            # ---- per-batch quadratic forms, 8 batches (4 slots) per group --
            # t buffers collect 4 groups before the partition-sum matmul
            GH = 4  # groups per half
            for h in range(n_groups // GH):
                tq0 = gpool.tile([128, GH * n_slots * 64], MM_DT, tag="t0")
                tq1 = gpool.tile([128, GH * n_slots * 64], MM_DT, tag="t1")
                tq = [tq0, tq1]
                for gi in range(GH):
                    g = h * GH + gi
                    slg = pspool.tile([128, n_slots * 128], F32, tag="fin")
                    for s_ in range(n_slots):
                        b0 = g * GRP + 2 * s_
                        nc.tensor.matmul(
                            slg[0:64, s_ * 128:(s_ + 1) * 128],
                            Lfin[:, b0 * 64:(b0 + 1) * 64], Pr[:, :, b0, :],
                            start=True, stop=True, tile_position=(0, 0),
                        )
                        nc.tensor.matmul(
                            slg[64:128, s_ * 128:(s_ + 1) * 128],
                            Lfin[:, (b0 + 1) * 64:(b0 + 2) * 64], Pr[:, :, b0 + 1, :],
                            start=True, stop=True, tile_position=(0, 64),
                        )
                    slg_r = slg[:].rearrange("p (s q j) -> p s q j", q=2, j=64)
                    srsb_r = SRr[:, g * n_slots:(g + 1) * n_slots, :]
                    for q in range(2):
                        t_r = tq[q][:, gi * n_slots * 64:(gi + 1) * n_slots * 64
                                     ].rearrange("p (s j) -> p s j", j=64)
                        nc.vector.tensor_mul(t_r, slg_r[:, :, q, :], srsb_r)
                for q in range(2):
                    zp = pspool1.tile([2, GH * n_slots * 64], F32, tag="zpb")
                    nc.tensor.matmul(
                        zp[:, 0:512], ones2[:], tq[q][:, 0:512],
                        start=True, stop=True,
                    )
                    nc.tensor.matmul(
                        zp[:, 512:1024], ones2[:], tq[q][:, 512:1024],
                        start=True, stop=True,
                    )
                    nc.vector.reduce_sum(
                        out=esb[0:2, q * 32 + h * GH * n_slots:
                                q * 32 + (h + 1) * GH * n_slots],
                        in_=zp[:].rearrange("p (g j) -> p g j", j=64),
                        axis=mybir.AxisListType.X,
                    )

            nc.sync.dma_start(out=out_d[:, :], in_=esb[:])

    nc.compile()
    return nc


def _get_nc():
    if "nc" not in _nc_cache:
        _nc_cache["nc"] = _build_nc()
    return _nc_cache["nc"]


# ----------------------------------------------------------------------------
# Entry point
# ----------------------------------------------------------------------------

def _decode_out(raw):
    """raw (2, 64): [parity, q*32 + g*4 + slot] -> (64, 2) e[b, q]."""
    e = np.empty((BPC, 2), dtype=np.float32)
    idx = np.arange(32)
    g, s_ = idx // 4, idx % 4
    b_even = g * 8 + s_ * 2
    for q in range(2):
        e[b_even, q] = raw[0, q * 32 + idx]
        e[b_even + 1, q] = raw[1, q * 32 + idx]
    return e


def kernel(X, theta, A, B, D, _trace=False):
    U, V, at_pack, bt_pack, G1r, lam = _host_prep(X, theta, A, B, D)
    np_mm = mybir.dt.np(MM_DT)
    at = np.ascontiguousarray(at_pack, dtype=np_mm)
    bt = np.ascontiguousarray(bt_pack, dtype=np_mm)
    g = np.ascontiguousarray(G1r, dtype=np_mm)
    lam_a = np.ascontiguousarray(lam.reshape(DA, 1), dtype=np.float32)
    in_maps = []
    for i in range(N_CORES):
        sl = slice(i * BPC, (i + 1) * BPC)
        in_maps.append(
            {
                "ut": np.ascontiguousarray(U[sl].T, dtype=np_mm),
                "vt": np.ascontiguousarray(V[sl].T, dtype=np_mm),
                "at": at,
                "bt": bt,
                "g": g,
                "lam": lam_a,
            }
        )
    nc = _get_nc()
    kw = {}
    if _trace:
        import shutil
        import tempfile

        shutil.rmtree("/tmp/vqc_prof", ignore_errors=True)
        os.makedirs("/tmp/vqc_prof", exist_ok=True)
        kw["tmpdir"] = tempfile.mkdtemp(dir="/tmp/vqc_prof")
    res = run_bass_kernel_spmd(nc, in_maps, list(range(N_CORES)), trace=_trace, **kw)
    outs = [_decode_out(res.results[i]["out"]) for i in range(N_CORES)]
    full = np.concatenate(outs, axis=0).astype(np.float32)
    if _trace:
        _nc_cache["last_exec_ns"] = res.exec_time_ns
        _nc_cache["last_results"] = res
    return full
